# revision 28
# baseline (speedup 1.0000x reference)
"""Trainium2 Bass kernel for nn_AGGate (emotion-conditioned attention gate).

Math (per batch b):
    v      = emotion @ W1 + b1 + b2                          (1, D) row
    logits = tanh(sents @ W2 + v) @ W3          (+b3: softmax-invariant)
    s      = softmax(logits)       c = cumsum(s)
    out    = (c - s) * pre + s * sents + (1 - c) * pos

Distribution: pure data-parallel over batch. B == 8 == n_cores, one batch
per NeuronCore, no collectives. Weights are replicated.

Reformulation that breaks the softmax barrier: with unnormalized E = exp(l),
running prefix c_raw, and total T (rden = 1/T),
    out = pos * (1 - c_raw*rden) + rden * U',
    U'  = (c_raw - E) * pre + E * sents,
so U' depends only on PREFIX data and is computed chunk-by-chunk, fully
pipelined with the logits matmuls, in place over the resident sents copy.

Per-core schedule (L = 8192, D = 512; 16 chunks of 512 rows):
  Phase 1 (PE-bound, ~140us): per chunk - stream sents row-major into a
    resident chunk tile; PE-transpose 128x128 blocks (3-deep PSUM pool);
    sq.T = W2.T @ X.T in float32r (TF32-class, operands rounded via the
    ACT psum->SBUF copies); tanh with per-partition bias v.T; logits via a
    second f32r matmul; logit rows bounce through DRAM into (128, q)
    column layout; E = exp; blocked prefix = triu-ones matmul + GpSimd
    partition-reduce totals + native VectorEngine scan chained across
    chunks; then U' in place (pre streamed, scaled in place).  All chain/U
    work is emitted LAG=2 chunks behind its logits to avoid head-of-line
    blocking on the in-order engines.
  Phase 2 (DMA-bound, ~110us): stream pos, scale in place by (1 - c*rden)
    (ACT/DVE alternating), add U'*rden (stt), store from the same tile.
    Loads issue on the sync queue, stores on the scalar queue so
    compute-gated store issues never block load prefetch.
"""

import sys
import types

import numpy as np

# ---------------------------------------------------------------------------
# Environment patches (self-contained; duplicated from the dev tree).
# ---------------------------------------------------------------------------

import antenv  # noqa: F401

if "antenv.axon_hooks" not in sys.modules:
    _mod = types.ModuleType("antenv.axon_hooks")
    _mod._hook = None
    _mod.set_axon_ntff_profile_hook = lambda h: setattr(_mod, "_hook", h)
    _mod.get_axon_ntff_profile_hook = lambda: _mod._hook
    sys.modules["antenv.axon_hooks"] = _mod
    try:
        from trn_agent_boot.trn_boot import _ntff_profile_via_ctypes

        _mod.set_axon_ntff_profile_hook(
            _ntff_profile_via_ctypes("/opt/axon/libaxon_pjrt.so")
        )
    except Exception:
        pass

import concourse.bass as bass
import concourse.bass_utils as bass_utils
import concourse.mybir as mybir
import concourse.tile as tile
from concourse.bass_utils import run_bass_kernel_spmd
from concourse.masks import make_identity
from concourse.vector_clock import ScopedClock

bass_utils.upload_artifacts = lambda tmpdir: tmpdir

_MAX_WAITS = 1
_spill_counter = [0]


def _split_waits_in_list(insts):
    out = []
    for inst in insts:
        si = getattr(inst, "sync_info", None)
        waits = list(si.on_wait) if si is not None and si.on_wait else []
        if len(waits) > _MAX_WAITS:
            si.on_wait = waits[-_MAX_WAITS:]
            for cs in range(0, len(waits) - _MAX_WAITS, _MAX_WAITS):
                chunk = waits[cs : cs + _MAX_WAITS]
                _spill_counter[0] += 1
                nop = mybir.InstNoOp(
                    name=f"I-waitspill-{_spill_counter[0]}", ins=[], outs=[]
                )
                nop.engine = inst.engine
                nop.sync_info = mybir.SyncInfo(on_wait=chunk, on_update=[])
                out.append(nop)
        out.append(inst)
    return out


_orig_lower = tile.TileContext._lower_ordered_insts


def _patched_lower(self, ordered):
    for bb_name in list(ordered.keys()):
        ordered[bb_name] = _split_waits_in_list(ordered[bb_name])
    return _orig_lower(self, ordered)


def _patched_drain_and_barrier(self, tick_clock, wait_clock):
    drain_inst = self.nc.sync.drain()
    wait_clock.add_sem_waits(
        drain_inst.ins, ScopedClock({None: tick_clock.global_clock})
    )
    si = drain_inst.ins.sync_info
    if si is not None and len(si.on_wait) > _MAX_WAITS:
        waits = list(si.on_wait)
        si.on_wait = waits[:_MAX_WAITS]
        rest = waits[_MAX_WAITS:]
        while rest:
            d2 = self.nc.sync.drain()
            d2.ins.sync_info = mybir.SyncInfo(
                on_wait=rest[:_MAX_WAITS], on_update=[]
            )
            rest = rest[_MAX_WAITS:]
    self.nc.all_engine_barrier()
    popped = self.nc._tile_sem_poison_stack.pop()
    assert popped is self._sem_poison
    self.nc.clear_and_free_semaphores(list(self.sems.allocated().values()))
    self.nc.all_engine_barrier()


tile.TileContext._lower_ordered_insts = _patched_lower
tile.TileContext._drain_and_barrier = _patched_drain_and_barrier

# ---------------------------------------------------------------------------
# Kernel builder
# ---------------------------------------------------------------------------

F32 = mybir.dt.float32
F32R = mybir.dt.float32r
ALU = mybir.AluOpType
ACTF = mybir.ActivationFunctionType

L, D = 8192, 512
NQ = L // 128  # 64 row-tiles of 128
NCH = 16  # phase-A chunks of 512 rows
QPC = 4  # row-tiles per phase-A chunk
NCHB = 32  # phase-B chunks of 256 rows
QPB = 2


def build_nc():
    nc = bass.Bass()

    sents = nc.dram_tensor("sents", [L, D], F32, kind="ExternalInput")
    pre = nc.dram_tensor("pre", [L, D], F32, kind="ExternalInput")
    pos = nc.dram_tensor("pos", [L, D], F32, kind="ExternalInput")
    emo = nc.dram_tensor("emotion", [1, D], F32, kind="ExternalInput")
    w1 = nc.dram_tensor("W1", [D, D], F32, kind="ExternalInput")
    w2 = nc.dram_tensor("W2", [D, D], F32, kind="ExternalInput")
    w3 = nc.dram_tensor("W3", [D, 1], F32, kind="ExternalInput")
    b1 = nc.dram_tensor("b1", [1, D], F32, kind="ExternalInput")
    b2 = nc.dram_tensor("b2", [1, D], F32, kind="ExternalInput")
    out = nc.dram_tensor("out", [L, D], F32, kind="ExternalOutput")

    lgdram = nc.dram_tensor("lgs", [1, L], F32)

    with tile.TileContext(nc) as tc:
        with (
            tc.tile_pool(name="consts", bufs=1) as consts,
            tc.tile_pool(name="resident", bufs=NCH) as resident,
            tc.tile_pool(name="xtp", bufs=2) as xtp,
            tc.tile_pool(name="ttp", bufs=1) as ttp,
            tc.tile_pool(name="rows", bufs=1) as rows,
            tc.tile_pool(name="small", bufs=5) as small,
            tc.tile_pool(name="iostr", bufs=2) as iostr,
            tc.tile_pool(name="posp", bufs=3) as posp,
            tc.tile_pool(name="trp", bufs=3, space="PSUM") as trp,
            tc.tile_pool(name="sqp", bufs=3, space="PSUM") as sqp,
            tc.tile_pool(name="lgp", bufs=1, space="PSUM") as lgp,
            tc.tile_pool(name="cump", bufs=1, space="PSUM") as cump,
        ):
            # ---- constants / setup -------------------------------------
            ident = consts.tile([128, 128], F32)
            make_identity(nc, ident)

            umat = consts.tile([128, 128], F32)  # umat[k, m] = 1 iff k <= m
            nc.gpsimd.memset(umat, 0.0)
            nc.gpsimd.affine_select(
                out=umat,
                in_=umat,
                compare_op=ALU.is_gt,
                fill=1.0,
                base=0,
                pattern=[[-1, 128]],
                channel_multiplier=1,
            )

            ones_col = consts.tile([128, 1], F32)
            nc.vector.memset(ones_col, 1.0)
            ones_row = consts.tile([1, 128], F32)
            nc.vector.memset(ones_row, 1.0)
            zrow = consts.tile([1, QPC], F32)
            nc.vector.memset(zrow, 0.0)

            w1s = xtp.tile([128, 4, D], F32, tag="xt")
            nc.scalar.dma_start(
                out=w1s, in_=w1[:, :].rearrange("(k p) n -> p k n", p=128)
            )
            w2s = xtp.tile([128, 4, D], F32, tag="xt")
            nc.scalar.dma_start(
                out=w2s, in_=w2[:, :].rearrange("(k p) n -> p k n", p=128)
            )
            w2r = consts.tile([128, 4, D], F32R)
            nc.vector.tensor_copy(out=w2r, in_=w2s)

            ecol = consts.tile([128, 4], F32)
            nc.sync.dma_start(
                out=ecol, in_=bass.AP(tensor=emo, offset=0, ap=[[1, 128], [128, 4]])
            )
            w3c = consts.tile([128, 4], F32)
            nc.sync.dma_start(
                out=w3c, in_=bass.AP(tensor=w3, offset=0, ap=[[1, 128], [128, 4]])
            )
            w3r = consts.tile([128, 4], F32R)
            nc.vector.tensor_copy(out=w3r, in_=w3c)
            b1c = consts.tile([128, 4], F32)
            nc.sync.dma_start(
                out=b1c, in_=bass.AP(tensor=b1, offset=0, ap=[[1, 128], [128, 4]])
            )
            b2c = consts.tile([128, 4], F32)
            nc.sync.dma_start(
                out=b2c, in_=bass.AP(tensor=b2, offset=0, ap=[[1, 128], [128, 4]])
            )
            bcol = consts.tile([128, 4], F32)
            nc.vector.tensor_tensor(out=bcol, in0=b1c, in1=b2c, op=ALU.add)

            vt = consts.tile([128, 4], F32)

            def emit_vt():
                vps = sqp.tile([128, 512], F32, tag="sq")
                for jn in range(4):
                    for k in range(4):
                        nc.tensor.matmul(
                            vps[:, jn : jn + 1],
                            lhsT=w1s[:, k, jn * 128 : (jn + 1) * 128],
                            rhs=ecol[:, k : k + 1],
                            start=(k == 0),
                            stop=(k == 3),
                        )
                nc.vector.tensor_tensor(
                    out=vt, in0=vps[:, 0:4], in1=bcol, op=ALU.add
                )

            craw_grid = consts.tile([128, NQ], F32)

            # ---- phase 1: logits + incremental prefix + U' ------------
            # U' = (craw - E) * pre + E * sents, written in place over the
            # resident sents chunk. pos is NOT touched in this phase.
            #
            # The softmax/prefix chain for a chunk has ~5us of cross-engine
            # latency (DRAM bounce + scan). Emitting it inline head-of-line
            # blocks the in-order engines and stalls the next chunk's PE
            # work, so chunk c's chain/U work is emitted LAG chunks behind
            # its logits work.
            LAG = 2
            sc_tiles = [None] * NCH
            state = {"incl_prev": None}

            def emit_logits(c, mid_hook=None):
                sc = resident.tile([128, QPC, D], F32, tag="res")
                sc_tiles[c] = sc
                nc.sync.dma_start(
                    out=sc,
                    in_=sents[c * 512 : (c + 1) * 512, :].rearrange(
                        "(g p) d -> p g d", p=128
                    ),
                )
                xt = xtp.tile([128, 4, D], F32R, tag="xt")
                for jq in range(QPC):
                    tpb = trp.tile([128, 512], F32, tag="tp")
                    for k in range(4):
                        nc.tensor.transpose(
                            tpb[:, k * 128 : (k + 1) * 128],
                            sc[:, jq, k * 128 : (k + 1) * 128],
                            ident,
                        )
                    nc.scalar.copy(
                        out=xt[:, :, jq * 128 : (jq + 1) * 128],
                        in_=tpb.rearrange("p (k j) -> p k j", j=128),
                    )
                if mid_hook is not None:
                    mid_hook()
                tt = ttp.tile([128, 4, D], F32R, tag="tt")
                for jn in range(4):
                    sq_ps = sqp.tile([128, 512], F32, tag="sq")
                    for k in range(4):
                        nc.tensor.matmul(
                            sq_ps,
                            lhsT=w2r[:, k, jn * 128 : (jn + 1) * 128],
                            rhs=xt[:, k, :],
                            start=(k == 0),
                            stop=(k == 3),
                        )
                    nc.scalar.activation(
                        out=tt[:, jn, :],
                        in_=sq_ps,
                        func=ACTF.Tanh,
                        bias=vt[:, jn : jn + 1],
                        scale=1.0,
                    )
                lg_ps = lgp.tile([1, 512], F32, tag="lg")
                for jn in range(4):
                    nc.tensor.matmul(
                        lg_ps,
                        lhsT=w3r[:, jn : jn + 1],
                        rhs=tt[:, jn, :],
                        start=(jn == 0),
                        stop=(jn == 3),
                    )
                lgrow = rows.tile([1, 512], F32, tag="lgrow")
                nc.vector.tensor_copy(out=lgrow, in_=lg_ps)
                nc.sync.dma_start(
                    out=lgdram[0:1, c * 512 : (c + 1) * 512], in_=lgrow
                )
                lgcols = small.tile([128, QPC], F32, tag="lgcols")
                nc.sync.dma_start(
                    out=lgcols,
                    in_=bass.AP(
                        tensor=lgdram,
                        offset=c * 512,
                        ap=[[1, 128], [128, QPC]],
                    ),
                )
                return lgcols

            def emit_chain(c, lgcols):
                e_c = small.tile([128, QPC], F32, tag="e")
                nc.scalar.activation(out=e_c, in_=lgcols, func=ACTF.Exp)

                cum_ps = cump.tile([128, QPC], F32, tag="cum")
                nc.tensor.matmul(cum_ps, lhsT=umat, rhs=e_c, start=True, stop=False)
                tot_c = small.tile([1, QPC], F32, tag="tot")
                nc.gpsimd.tensor_reduce(
                    out=tot_c, in_=e_c, axis=mybir.AxisListType.C, op=ALU.add
                )
                incl = small.tile([1, QPC], F32, tag="incl")
                nc.vector.tensor_tensor_scan(
                    out=incl,
                    data0=tot_c,
                    data1=zrow,
                    initial=(
                        0.0
                        if state["incl_prev"] is None
                        else state["incl_prev"][:, QPC - 1 : QPC]
                    ),
                    op0=ALU.add,
                    op1=ALU.add,
                )
                state["incl_prev"] = incl
                off = small.tile([1, QPC], F32, tag="off")
                nc.vector.tensor_tensor(
                    out=off, in0=incl, in1=tot_c, op=ALU.subtract
                )
                nc.tensor.matmul(cum_ps, lhsT=ones_row, rhs=off, start=False, stop=True)
                craw_c = craw_grid[:, QPC * c : QPC * (c + 1)]
                nc.vector.tensor_copy(out=craw_c, in_=cum_ps)
                cme = small.tile([128, QPC], F32, tag="cme")
                nc.vector.tensor_tensor(
                    out=cme, in0=craw_c, in1=e_c, op=ALU.subtract
                )

                sc = sc_tiles[c]
                pr = iostr.tile([128, QPC, D], F32, tag="pre")
                nc.sync.dma_start(
                    out=pr[:, 0:2, :],
                    in_=pre[c * 512 : c * 512 + 256, :].rearrange(
                        "(g p) d -> p g d", p=128
                    ),
                )
                nc.sync.dma_start(
                    out=pr[:, 2:4, :],
                    in_=pre[c * 512 + 256 : (c + 1) * 512, :].rearrange(
                        "(g p) d -> p g d", p=128
                    ),
                )
                for jq in range(QPC):
                    # pr <- pre * (craw - E)   (in place; alternate engines)
                    if jq % 2 == 0:
                        nc.vector.tensor_scalar_mul(
                            out=pr[:, jq, :],
                            in0=pr[:, jq, :],
                            scalar1=cme[:, jq : jq + 1],
                        )
                    else:
                        nc.scalar.activation(
                            out=pr[:, jq, :],
                            in_=pr[:, jq, :],
                            func=ACTF.Copy,
                            bias=0.0,
                            scale=cme[:, jq : jq + 1],
                        )
                    # sc <- sents * E + pr     (in place over sents -> U')
                    nc.vector.scalar_tensor_tensor(
                        out=sc[:, jq, :],
                        in0=sc[:, jq, :],
                        scalar=e_c[:, jq : jq + 1],
                        in1=pr[:, jq, :],
                        op0=ALU.mult,
                        op1=ALU.add,
                    )

            pending = []
            for c in range(NCH):
                pending.append(
                    (c, emit_logits(c, mid_hook=emit_vt if c == 0 else None))
                )
                if len(pending) > LAG:
                    cc, lgc = pending.pop(0)
                    emit_chain(cc, lgc)
            for cc, lgc in pending:
                emit_chain(cc, lgc)
            incl_prev = state["incl_prev"]

            # ---- normalization scalars --------------------------------
            rden = consts.tile([1, 1], F32)
            nc.vector.reciprocal(out=rden, in_=incl_prev[:, QPC - 1 : QPC])
            rdb = sqp.tile([128, QPC], F32, tag="sq")
            nc.tensor.matmul(
                rdb[:, 0:1], lhsT=ones_row, rhs=rden, start=True, stop=True
            )
            rden_col = consts.tile([128, 1], F32)
            nc.vector.tensor_copy(out=rden_col, in_=rdb[:, 0:1])
            cn_grid = consts.tile([128, NQ], F32)
            nc.vector.tensor_scalar_mul(
                out=cn_grid, in0=craw_grid, scalar1=rden_col
            )
            posw = consts.tile([128, NQ], F32)  # 1 - c
            nc.scalar.activation(
                out=posw, in_=cn_grid, func=ACTF.Copy, bias=1.0, scale=-1.0
            )

            # ---- phase 2: out = pos * (1 - c) + U' * rden -------------
            for h in range(NCH):
                r0 = h * 512
                r1 = (h + 1) * 512
                po = posp.tile([128, QPC, D], F32, tag="pos")
                nc.sync.dma_start(
                    out=po,
                    in_=pos[r0:r1, :].rearrange("(g p) d -> p g d", p=128),
                )
                sc = sc_tiles[h]
                for jq in range(QPC):
                    q = QPC * h + jq
                    # po <- pos * (1 - c)      (in place; alternate engines)
                    if jq % 2 == 0:
                        nc.scalar.activation(
                            out=po[:, jq, :],
                            in_=po[:, jq, :],
                            func=ACTF.Copy,
                            bias=0.0,
                            scale=posw[:, q : q + 1],
                        )
                    else:
                        nc.vector.tensor_scalar_mul(
                            out=po[:, jq, :],
                            in0=po[:, jq, :],
                            scalar1=posw[:, q : q + 1],
                        )
                    # po <- U' * rden + po     (VectorEngine, in place)
                    nc.vector.scalar_tensor_tensor(
                        out=po[:, jq, :],
                        in0=sc[:, jq, :],
                        scalar=rden_col,
                        in1=po[:, jq, :],
                        op0=ALU.mult,
                        op1=ALU.add,
                    )
                nc.scalar.dma_start(
                    out=out[r0:r1, :].rearrange("(g p) d -> p g d", p=128),
                    in_=po,
                )

    return nc


_NC_CACHE = None


def _get_nc():
    global _NC_CACHE
    if _NC_CACHE is None:
        _NC_CACHE = build_nc()
    return _NC_CACHE


def kernel(**inputs) -> np.ndarray:
    emotion_h = np.asarray(inputs["emotion_h"], np.float32)
    sents_h = np.asarray(inputs["sents_h"], np.float32)
    pre_h = np.asarray(inputs["pre_sents_h"], np.float32)
    pos_h = np.asarray(inputs["pos_sents_h"], np.float32)
    W1 = np.ascontiguousarray(inputs["W1"], np.float32)
    W2 = np.ascontiguousarray(inputs["W2"], np.float32)
    W3 = np.ascontiguousarray(inputs["W3"], np.float32).reshape(D, 1)
    b1 = np.ascontiguousarray(inputs["b1"], np.float32).reshape(1, D)
    b2 = np.ascontiguousarray(inputs["b2"], np.float32).reshape(1, D)

    B = sents_h.shape[0]
    assert B == 8, f"expected B=8, got {B}"

    in_maps = []
    for i in range(B):
        in_maps.append(
            {
                "sents": np.ascontiguousarray(sents_h[i]),
                "pre": np.ascontiguousarray(pre_h[i]),
                "pos": np.ascontiguousarray(pos_h[i]),
                "emotion": np.ascontiguousarray(emotion_h[i].reshape(1, D)),
                "W1": W1,
                "W2": W2,
                "W3": W3,
                "b1": b1,
                "b2": b2,
            }
        )

    nc = _get_nc()
    res = run_bass_kernel_spmd(nc, in_maps, core_ids=list(range(8)))
    return np.stack([res.results[i]["out"] for i in range(B)]).astype(np.float32)


# revision 29
# speedup vs baseline: 1.1260x; 1.1260x over previous
"""Trainium2 Bass kernel for nn_AGGate (emotion-conditioned attention gate).

Math (per batch b):
    v      = emotion @ W1 + b1 + b2                          (1, D) row
    logits = tanh(sents @ W2 + v) @ W3          (+b3: softmax-invariant)
    s      = softmax(logits)       c = cumsum(s)
    out    = (c - s) * pre + s * sents + (1 - c) * pos

Distribution: pure data-parallel over batch. B == 8 == n_cores, one batch
per NeuronCore, no collectives. Weights are replicated.

Reformulation that breaks the softmax barrier: with unnormalized E = exp(l),
running prefix c_raw, and total T (rden = 1/T),
    out = pos * (1 - c_raw*rden) + rden * U',
    U'  = (c_raw - E) * pre + E * sents,
so U' depends only on PREFIX data and is computed chunk-by-chunk, fully
pipelined with the logits matmuls, in place over the resident sents copy.

Per-core schedule (L = 8192, D = 512; 16 chunks of 512 rows):
  Phase 1 (PE-bound, ~140us): per chunk - stream sents row-major into a
    resident chunk tile; PE-transpose 128x128 blocks (3-deep PSUM pool);
    sq.T = W2.T @ X.T in float32r (TF32-class, operands rounded via the
    ACT psum->SBUF copies); tanh with per-partition bias v.T; logits via a
    second f32r matmul; logit rows bounce through DRAM into (128, q)
    column layout; E = exp; blocked prefix = triu-ones matmul + GpSimd
    partition-reduce totals + native VectorEngine scan chained across
    chunks; then U' in place (pre streamed, scaled in place).  All chain/U
    work is emitted LAG=2 chunks behind its logits to avoid head-of-line
    blocking on the in-order engines.
  Phase 2 (DMA-bound, ~110us): stream pos, scale in place by (1 - c*rden)
    (ACT/DVE alternating), add U'*rden (stt), store from the same tile.
    Loads issue on the sync queue, stores on the scalar queue so
    compute-gated store issues never block load prefetch.
"""

import sys
import types

import numpy as np

# ---------------------------------------------------------------------------
# Environment patches (self-contained; duplicated from the dev tree).
# ---------------------------------------------------------------------------

import antenv  # noqa: F401

if "antenv.axon_hooks" not in sys.modules:
    _mod = types.ModuleType("antenv.axon_hooks")
    _mod._hook = None
    _mod.set_axon_ntff_profile_hook = lambda h: setattr(_mod, "_hook", h)
    _mod.get_axon_ntff_profile_hook = lambda: _mod._hook
    sys.modules["antenv.axon_hooks"] = _mod
    try:
        from trn_agent_boot.trn_boot import _ntff_profile_via_ctypes

        _mod.set_axon_ntff_profile_hook(
            _ntff_profile_via_ctypes("/opt/axon/libaxon_pjrt.so")
        )
    except Exception:
        pass

import concourse.bass as bass
import concourse.bass_utils as bass_utils
import concourse.mybir as mybir
import concourse.tile as tile
from concourse.bass_utils import run_bass_kernel_spmd
from concourse.masks import make_identity
from concourse.vector_clock import ScopedClock

bass_utils.upload_artifacts = lambda tmpdir: tmpdir

_MAX_WAITS = 1
_spill_counter = [0]


def _split_waits_in_list(insts):
    out = []
    for inst in insts:
        si = getattr(inst, "sync_info", None)
        waits = list(si.on_wait) if si is not None and si.on_wait else []
        if len(waits) > _MAX_WAITS:
            si.on_wait = waits[-_MAX_WAITS:]
            for cs in range(0, len(waits) - _MAX_WAITS, _MAX_WAITS):
                chunk = waits[cs : cs + _MAX_WAITS]
                _spill_counter[0] += 1
                nop = mybir.InstNoOp(
                    name=f"I-waitspill-{_spill_counter[0]}", ins=[], outs=[]
                )
                nop.engine = inst.engine
                nop.sync_info = mybir.SyncInfo(on_wait=chunk, on_update=[])
                out.append(nop)
        out.append(inst)
    return out


_orig_lower = tile.TileContext._lower_ordered_insts


def _patched_lower(self, ordered):
    for bb_name in list(ordered.keys()):
        ordered[bb_name] = _split_waits_in_list(ordered[bb_name])
    return _orig_lower(self, ordered)


def _patched_drain_and_barrier(self, tick_clock, wait_clock):
    drain_inst = self.nc.sync.drain()
    wait_clock.add_sem_waits(
        drain_inst.ins, ScopedClock({None: tick_clock.global_clock})
    )
    si = drain_inst.ins.sync_info
    if si is not None and len(si.on_wait) > _MAX_WAITS:
        waits = list(si.on_wait)
        si.on_wait = waits[:_MAX_WAITS]
        rest = waits[_MAX_WAITS:]
        while rest:
            d2 = self.nc.sync.drain()
            d2.ins.sync_info = mybir.SyncInfo(
                on_wait=rest[:_MAX_WAITS], on_update=[]
            )
            rest = rest[_MAX_WAITS:]
    self.nc.all_engine_barrier()
    popped = self.nc._tile_sem_poison_stack.pop()
    assert popped is self._sem_poison
    self.nc.clear_and_free_semaphores(list(self.sems.allocated().values()))
    self.nc.all_engine_barrier()


tile.TileContext._lower_ordered_insts = _patched_lower
tile.TileContext._drain_and_barrier = _patched_drain_and_barrier

# ---------------------------------------------------------------------------
# Kernel builder
# ---------------------------------------------------------------------------

F32 = mybir.dt.float32
F32R = mybir.dt.float32r
ALU = mybir.AluOpType
ACTF = mybir.ActivationFunctionType

L, D = 8192, 512
NQ = L // 128  # 64 row-tiles of 128
NCH = 16  # phase-A chunks of 512 rows
QPC = 4  # row-tiles per phase-A chunk
NCHB = 32  # phase-B chunks of 256 rows
QPB = 2


def build_nc():
    nc = bass.Bass()

    sents = nc.dram_tensor("sents", [L, D], F32, kind="ExternalInput")
    pre = nc.dram_tensor("pre", [L, D], F32, kind="ExternalInput")
    pos = nc.dram_tensor("pos", [L, D], F32, kind="ExternalInput")
    emo = nc.dram_tensor("emotion", [1, D], F32, kind="ExternalInput")
    w1 = nc.dram_tensor("W1", [D, D], F32, kind="ExternalInput")
    w2 = nc.dram_tensor("W2", [D, D], F32, kind="ExternalInput")
    w3 = nc.dram_tensor("W3", [D, 1], F32, kind="ExternalInput")
    b1 = nc.dram_tensor("b1", [1, D], F32, kind="ExternalInput")
    b2 = nc.dram_tensor("b2", [1, D], F32, kind="ExternalInput")
    out = nc.dram_tensor("out", [L, D], F32, kind="ExternalOutput")

    lgdram = nc.dram_tensor("lgs", [1, L], F32)

    with tile.TileContext(nc) as tc:
        with (
            tc.tile_pool(name="consts", bufs=1) as consts,
            tc.tile_pool(name="resident", bufs=NCH) as resident,
            tc.tile_pool(name="xtp", bufs=2) as xtp,
            tc.tile_pool(name="ttp", bufs=1) as ttp,
            tc.tile_pool(name="rows", bufs=1) as rows,
            tc.tile_pool(name="small", bufs=5) as small,
            tc.tile_pool(name="iostr", bufs=2) as iostr,
            tc.tile_pool(name="posp", bufs=6) as posp,
            tc.tile_pool(name="trp", bufs=3, space="PSUM") as trp,
            tc.tile_pool(name="sqp", bufs=3, space="PSUM") as sqp,
            tc.tile_pool(name="lgp", bufs=1, space="PSUM") as lgp,
            tc.tile_pool(name="cump", bufs=1, space="PSUM") as cump,
        ):
            # ---- constants / setup -------------------------------------
            ident = consts.tile([128, 128], F32)
            make_identity(nc, ident)

            umat = consts.tile([128, 128], F32)  # umat[k, m] = 1 iff k <= m
            nc.gpsimd.memset(umat, 0.0)
            nc.gpsimd.affine_select(
                out=umat,
                in_=umat,
                compare_op=ALU.is_gt,
                fill=1.0,
                base=0,
                pattern=[[-1, 128]],
                channel_multiplier=1,
            )

            ones_col = consts.tile([128, 1], F32)
            nc.vector.memset(ones_col, 1.0)
            ones_row = consts.tile([1, 128], F32)
            nc.vector.memset(ones_row, 1.0)
            zrow = consts.tile([1, QPC], F32)
            nc.vector.memset(zrow, 0.0)

            w1s = xtp.tile([128, 4, D], F32, tag="xt")
            nc.scalar.dma_start(
                out=w1s, in_=w1[:, :].rearrange("(k p) n -> p k n", p=128)
            )
            w2s = xtp.tile([128, 4, D], F32, tag="xt")
            nc.scalar.dma_start(
                out=w2s, in_=w2[:, :].rearrange("(k p) n -> p k n", p=128)
            )
            w2r = consts.tile([128, 4, D], F32R)
            nc.vector.tensor_copy(out=w2r, in_=w2s)

            ecol = consts.tile([128, 4], F32)
            nc.sync.dma_start(
                out=ecol, in_=bass.AP(tensor=emo, offset=0, ap=[[1, 128], [128, 4]])
            )
            w3c = consts.tile([128, 4], F32)
            nc.sync.dma_start(
                out=w3c, in_=bass.AP(tensor=w3, offset=0, ap=[[1, 128], [128, 4]])
            )
            w3r = consts.tile([128, 4], F32R)
            nc.vector.tensor_copy(out=w3r, in_=w3c)
            b1c = consts.tile([128, 4], F32)
            nc.sync.dma_start(
                out=b1c, in_=bass.AP(tensor=b1, offset=0, ap=[[1, 128], [128, 4]])
            )
            b2c = consts.tile([128, 4], F32)
            nc.sync.dma_start(
                out=b2c, in_=bass.AP(tensor=b2, offset=0, ap=[[1, 128], [128, 4]])
            )
            bcol = consts.tile([128, 4], F32)
            nc.vector.tensor_tensor(out=bcol, in0=b1c, in1=b2c, op=ALU.add)

            vt = consts.tile([128, 4], F32)

            def emit_vt():
                vps = sqp.tile([128, 512], F32, tag="sq")
                for jn in range(4):
                    for k in range(4):
                        nc.tensor.matmul(
                            vps[:, jn : jn + 1],
                            lhsT=w1s[:, k, jn * 128 : (jn + 1) * 128],
                            rhs=ecol[:, k : k + 1],
                            start=(k == 0),
                            stop=(k == 3),
                        )
                nc.vector.tensor_tensor(
                    out=vt, in0=vps[:, 0:4], in1=bcol, op=ALU.add
                )

            craw_grid = consts.tile([128, NQ], F32)

            # ---- phase 1: logits + incremental prefix + U' ------------
            # U' = (craw - E) * pre + E * sents, written in place over the
            # resident sents chunk. pos is NOT touched in this phase.
            #
            # The softmax/prefix chain for a chunk has ~5us of cross-engine
            # latency (DRAM bounce + scan). Emitting it inline head-of-line
            # blocks the in-order engines and stalls the next chunk's PE
            # work, so chunk c's chain/U work is emitted LAG chunks behind
            # its logits work.
            LAG = 2
            sc_tiles = [None] * NCH
            state = {"incl_prev": None}

            def emit_logits(c, mid_hook=None):
                sc = resident.tile([128, QPC, D], F32, tag="res")
                sc_tiles[c] = sc
                nc.sync.dma_start(
                    out=sc,
                    in_=sents[c * 512 : (c + 1) * 512, :].rearrange(
                        "(g p) d -> p g d", p=128
                    ),
                )
                xt = xtp.tile([128, 4, D], F32R, tag="xt")
                for jq in range(QPC):
                    tpb = trp.tile([128, 512], F32, tag="tp")
                    for k in range(4):
                        nc.tensor.transpose(
                            tpb[:, k * 128 : (k + 1) * 128],
                            sc[:, jq, k * 128 : (k + 1) * 128],
                            ident,
                        )
                    nc.scalar.copy(
                        out=xt[:, :, jq * 128 : (jq + 1) * 128],
                        in_=tpb.rearrange("p (k j) -> p k j", j=128),
                    )
                if mid_hook is not None:
                    mid_hook()
                tt = ttp.tile([128, 4, D], F32R, tag="tt")
                for jn in range(4):
                    sq_ps = sqp.tile([128, 512], F32, tag="sq")
                    for k in range(4):
                        nc.tensor.matmul(
                            sq_ps,
                            lhsT=w2r[:, k, jn * 128 : (jn + 1) * 128],
                            rhs=xt[:, k, :],
                            start=(k == 0),
                            stop=(k == 3),
                        )
                    nc.scalar.activation(
                        out=tt[:, jn, :],
                        in_=sq_ps,
                        func=ACTF.Tanh,
                        bias=vt[:, jn : jn + 1],
                        scale=1.0,
                    )
                lg_ps = lgp.tile([1, 512], F32, tag="lg")
                for jn in range(4):
                    nc.tensor.matmul(
                        lg_ps,
                        lhsT=w3r[:, jn : jn + 1],
                        rhs=tt[:, jn, :],
                        start=(jn == 0),
                        stop=(jn == 3),
                    )
                lgrow = rows.tile([1, 512], F32, tag="lgrow")
                nc.vector.tensor_copy(out=lgrow, in_=lg_ps)
                nc.sync.dma_start(
                    out=lgdram[0:1, c * 512 : (c + 1) * 512], in_=lgrow
                )
                lgcols = small.tile([128, QPC], F32, tag="lgcols")
                nc.sync.dma_start(
                    out=lgcols,
                    in_=bass.AP(
                        tensor=lgdram,
                        offset=c * 512,
                        ap=[[1, 128], [128, QPC]],
                    ),
                )
                return lgcols

            def emit_chain(c, lgcols):
                e_c = small.tile([128, QPC], F32, tag="e")
                nc.scalar.activation(out=e_c, in_=lgcols, func=ACTF.Exp)

                cum_ps = cump.tile([128, QPC], F32, tag="cum")
                nc.tensor.matmul(cum_ps, lhsT=umat, rhs=e_c, start=True, stop=False)
                tot_c = small.tile([1, QPC], F32, tag="tot")
                nc.gpsimd.tensor_reduce(
                    out=tot_c, in_=e_c, axis=mybir.AxisListType.C, op=ALU.add
                )
                incl = small.tile([1, QPC], F32, tag="incl")
                nc.vector.tensor_tensor_scan(
                    out=incl,
                    data0=tot_c,
                    data1=zrow,
                    initial=(
                        0.0
                        if state["incl_prev"] is None
                        else state["incl_prev"][:, QPC - 1 : QPC]
                    ),
                    op0=ALU.add,
                    op1=ALU.add,
                )
                state["incl_prev"] = incl
                off = small.tile([1, QPC], F32, tag="off")
                nc.vector.tensor_tensor(
                    out=off, in0=incl, in1=tot_c, op=ALU.subtract
                )
                nc.tensor.matmul(cum_ps, lhsT=ones_row, rhs=off, start=False, stop=True)
                craw_c = craw_grid[:, QPC * c : QPC * (c + 1)]
                nc.vector.tensor_copy(out=craw_c, in_=cum_ps)
                cme = small.tile([128, QPC], F32, tag="cme")
                nc.vector.tensor_tensor(
                    out=cme, in0=craw_c, in1=e_c, op=ALU.subtract
                )

                sc = sc_tiles[c]
                pr = iostr.tile([128, QPC, D], F32, tag="pre")
                nc.sync.dma_start(
                    out=pr[:, 0:2, :],
                    in_=pre[c * 512 : c * 512 + 256, :].rearrange(
                        "(g p) d -> p g d", p=128
                    ),
                )
                nc.sync.dma_start(
                    out=pr[:, 2:4, :],
                    in_=pre[c * 512 + 256 : (c + 1) * 512, :].rearrange(
                        "(g p) d -> p g d", p=128
                    ),
                )
                for jq in range(QPC):
                    # pr <- pre * (craw - E)   (in place; alternate engines)
                    if jq % 2 == 0:
                        nc.vector.tensor_scalar_mul(
                            out=pr[:, jq, :],
                            in0=pr[:, jq, :],
                            scalar1=cme[:, jq : jq + 1],
                        )
                    else:
                        nc.scalar.activation(
                            out=pr[:, jq, :],
                            in_=pr[:, jq, :],
                            func=ACTF.Copy,
                            bias=0.0,
                            scale=cme[:, jq : jq + 1],
                        )
                    # sc <- sents * E + pr     (in place over sents -> U')
                    nc.vector.scalar_tensor_tensor(
                        out=sc[:, jq, :],
                        in0=sc[:, jq, :],
                        scalar=e_c[:, jq : jq + 1],
                        in1=pr[:, jq, :],
                        op0=ALU.mult,
                        op1=ALU.add,
                    )

            pending = []
            for c in range(NCH):
                pending.append(
                    (c, emit_logits(c, mid_hook=emit_vt if c == 0 else None))
                )
                if len(pending) > LAG:
                    cc, lgc = pending.pop(0)
                    emit_chain(cc, lgc)
            for cc, lgc in pending:
                emit_chain(cc, lgc)
            incl_prev = state["incl_prev"]

            # ---- normalization scalars --------------------------------
            rden = consts.tile([1, 1], F32)
            nc.vector.reciprocal(out=rden, in_=incl_prev[:, QPC - 1 : QPC])
            rdb = sqp.tile([128, QPC], F32, tag="sq")
            nc.tensor.matmul(
                rdb[:, 0:1], lhsT=ones_row, rhs=rden, start=True, stop=True
            )
            rden_col = consts.tile([128, 1], F32)
            nc.vector.tensor_copy(out=rden_col, in_=rdb[:, 0:1])
            cn_grid = consts.tile([128, NQ], F32)
            nc.vector.tensor_scalar_mul(
                out=cn_grid, in0=craw_grid, scalar1=rden_col
            )
            posw = consts.tile([128, NQ], F32)  # 1 - c
            nc.scalar.activation(
                out=posw, in_=cn_grid, func=ACTF.Copy, bias=1.0, scale=-1.0
            )

            # ---- phase 2: out = pos * (1 - c) + U' * rden -------------
            for h in range(NCHB):
                r0 = h * 128 * QPB
                r1 = (h + 1) * 128 * QPB
                po = posp.tile([128, QPB, D], F32, tag="pos")
                nc.sync.dma_start(
                    out=po,
                    in_=pos[r0:r1, :].rearrange("(g p) d -> p g d", p=128),
                )
                sc = sc_tiles[h // 2]
                for jq in range(QPB):
                    q = QPB * h + jq
                    # po <- pos * (1 - c)      (in place; alternate engines)
                    if jq % 2 == 0:
                        nc.scalar.activation(
                            out=po[:, jq, :],
                            in_=po[:, jq, :],
                            func=ACTF.Copy,
                            bias=0.0,
                            scale=posw[:, q : q + 1],
                        )
                    else:
                        nc.vector.tensor_scalar_mul(
                            out=po[:, jq, :],
                            in0=po[:, jq, :],
                            scalar1=posw[:, q : q + 1],
                        )
                    # po <- U' * rden + po     (VectorEngine, in place)
                    nc.vector.scalar_tensor_tensor(
                        out=po[:, jq, :],
                        in0=sc[:, (h % 2) * QPB + jq, :],
                        scalar=rden_col,
                        in1=po[:, jq, :],
                        op0=ALU.mult,
                        op1=ALU.add,
                    )
                nc.scalar.dma_start(
                    out=out[r0:r1, :].rearrange("(g p) d -> p g d", p=128),
                    in_=po,
                )

    return nc


_NC_CACHE = None


def _get_nc():
    global _NC_CACHE
    if _NC_CACHE is None:
        _NC_CACHE = build_nc()
    return _NC_CACHE


def kernel(**inputs) -> np.ndarray:
    emotion_h = np.asarray(inputs["emotion_h"], np.float32)
    sents_h = np.asarray(inputs["sents_h"], np.float32)
    pre_h = np.asarray(inputs["pre_sents_h"], np.float32)
    pos_h = np.asarray(inputs["pos_sents_h"], np.float32)
    W1 = np.ascontiguousarray(inputs["W1"], np.float32)
    W2 = np.ascontiguousarray(inputs["W2"], np.float32)
    W3 = np.ascontiguousarray(inputs["W3"], np.float32).reshape(D, 1)
    b1 = np.ascontiguousarray(inputs["b1"], np.float32).reshape(1, D)
    b2 = np.ascontiguousarray(inputs["b2"], np.float32).reshape(1, D)

    B = sents_h.shape[0]
    assert B == 8, f"expected B=8, got {B}"

    in_maps = []
    for i in range(B):
        in_maps.append(
            {
                "sents": np.ascontiguousarray(sents_h[i]),
                "pre": np.ascontiguousarray(pre_h[i]),
                "pos": np.ascontiguousarray(pos_h[i]),
                "emotion": np.ascontiguousarray(emotion_h[i].reshape(1, D)),
                "W1": W1,
                "W2": W2,
                "W3": W3,
                "b1": b1,
                "b2": b2,
            }
        )

    nc = _get_nc()
    res = run_bass_kernel_spmd(nc, in_maps, core_ids=list(range(8)))
    return np.stack([res.results[i]["out"] for i in range(B)]).astype(np.float32)
